# revision 53
# baseline (speedup 1.0000x reference)
"""Trainium2 Bass kernel for EquivariantAttention (sparse_attention).

Full (unsharded) inputs in, full output out. Internally shards over the 8
NeuronCores as (batch, T-half): core c handles batch b = c // 2, query rows
t0 = (c % 2) * 256 .. t0+256.  Every core runs the identical SPMD program on
its own input slices; there is no cross-core communication (LN and out_proj
are row-local in (b, t)).

v5 design (all-f16 datapath, software-pipelined):
  Per (head-group hg of 4 heads, s-tile st of 128):
    scores_T[s,t] = bias_T + k_T.T @ q_T     (bias preloaded into PSUM via
                                              identity matmul @ f16 full rate,
                                              QK accumulates on top)
    m = (scores_T + 20) * law                (DVE stt, f32 - only PSUM reader)
    e = exp(m - 20) -> f16                   (ACT, per-partition bias)
    den[t] += sum_s e                        (ones-column matmul)
    g = e * law4 (flat, no broadcast AP)     (f16; alternate DVE/Pool)
    numer[j,t] += sum_s v[s,j] * g[s,t]      (f16 matmul; PBC rows of V
                                              gathered on device)
  The loop is software-pipelined one deep: preload/QK/elementwise of iter n+1
  issue before den/numer of iter n so the PE never waits on the DVE/ACT chain.
  Per-hg drain (overlapped with the next hg's loop):
    rec = recip(den)  (pair 1 bounced via ACT copy: custom DVE ops misread
                       PSUM at partition offset 32)
    rb = partition_broadcast(rec);  anum = ACT copy of numer -> f16
    adiv = anum*rb (Pool);  sq = ACT square;  transposed ssq ones-matmuls
    remap adiv -> channel-major attn_ct[hg] via 12 small SBUF DMAs
  Tail: inorm = rsqrt(ssq/512+eps) directly in column form; out_proj f16
  matmuls with inorm folded into the PSUM->SBUF ACT copy.
"""

import numpy as np

import concourse.bass as bass
import concourse.bacc as bacc
import concourse.tile as tile
from concourse import mybir
from concourse.bass_utils import run_bass_kernel_spmd

# Problem constants (hardcoded per contract)
B, T, P, HID = 4, 512, 3, 512
H, D = 16, 32
EXP = 256
S = T + EXP            # 768
SCALING = (D / 3.0) ** 0.5 / D
SMOOTH = 20.0
EPS = 1e-3

NCORES = 8
TQ = T // 2            # 256 query rows per core
DH = P * D             # 96 head dim
NST = S // 128         # 6 s-tiles of 128
HG = 4                 # head groups of 4 heads
NIT = HG * NST         # 24 pipelined iterations

F32 = mybir.dt.float32
F16 = mybir.dt.float16
I32 = mybir.dt.int32
AF = mybir.ActivationFunctionType
ALU = mybir.AluOpType

_CACHED_NC = None
DEBUG_DUMPS = False


def build_nc():
    nc = bacc.Bacc("TRN2", target_bir_lowering=False, debug=False)

    # ---- DRAM I/O (per-core shapes) ----
    d_bias = nc.dram_tensor("biasT", [NIT * 128, 4 * TQ], F16,
                            kind="ExternalInput").ap()
    d_law4 = nc.dram_tensor("law4", [128, NST * 4 * TQ], F16,
                            kind="ExternalInput").ap()
    d_rec = nc.dram_tensor("rec_scratch", [8, 2 * TQ], F32).ap()
    d_qT = nc.dram_tensor("qT", [DH, H * TQ], F16, kind="ExternalInput").ap()
    d_kTe = nc.dram_tensor("kTe", [DH, H * S], F16, kind="ExternalInput").ap()
    d_vb = nc.dram_tensor("vb", [T, P * HID], F16, kind="ExternalInput").ap()
    d_vidx = nc.dram_tensor("vidx", [2, 128, 1], I32, kind="ExternalInput").ap()
    d_wT = nc.dram_tensor("wT", [128, 4 * HID], F16, kind="ExternalInput").ap()
    d_id = nc.dram_tensor("ident", [128, 128], F16, kind="ExternalInput").ap()
    d_out = nc.dram_tensor("out", [TQ, P, HID], F32, kind="ExternalOutput").ap()
    d_dbg = None
    if DEBUG_DUMPS:
        d_dbg = dict(
            attn=nc.dram_tensor("dbg_attn", [4 * 128, P * TQ], F16,
                                kind="ExternalOutput").ap(),
            e=nc.dram_tensor("dbg_e", [128, 4 * TQ], F16,
                             kind="ExternalOutput").ap(),
            g=nc.dram_tensor("dbg_g", [128, 4 * TQ], F16,
                             kind="ExternalOutput").ap(),
        )

    with tile.TileContext(nc) as tc:
        build_kernel(tc, d_bias, d_law4, d_qT, d_kTe, d_vb, d_vidx,
                     d_wT, d_id, d_out, d_rec, d_dbg)
    nc.compile()
    return nc


def build_kernel(tc, d_bias, d_law4, d_qT, d_kTe, d_vb, d_vidx,
                 d_wT, d_id, d_out, d_rec, d_dbg=None):
    nc = tc.nc
    from contextlib import ExitStack, nullcontext
    ctx = ExitStack()
    with ctx:
        const = ctx.enter_context(tc.tile_pool(name="const", bufs=1))
        big = ctx.enter_context(tc.tile_pool(name="big", bufs=1))
        biasp = ctx.enter_context(tc.tile_pool(name="biasp", bufs=6))
        work = ctx.enter_context(tc.tile_pool(name="work", bufs=3))
        drain = ctx.enter_context(tc.tile_pool(name="drain", bufs=2))
        attnp = ctx.enter_context(tc.tile_pool(name="attnp", bufs=1))
        psum = ctx.enter_context(tc.tile_pool(name="psum", bufs=2, space="PSUM"))
        psum1 = ctx.enter_context(tc.tile_pool(name="psum1", bufs=1, space="PSUM"))

        # ---- constants ----
        ones_h = const.tile([128, 1], F16, tag="ones_h")
        nc.vector.memset(ones_h[:], 1.0)
        # touch the Pool engine once so its tensor-op firmware library loads
        # during DMA warmup instead of stalling the first g-multiply (~5.6us)
        poolwarm = const.tile([128, 1], F16, tag="poolwarm")
        nc.gpsimd.tensor_tensor(out=poolwarm[:], in0=ones_h[:],
                                in1=ones_h[:], op=ALU.mult)
        ones_f = const.tile([128, 1], F32, tag="ones_f")
        nc.vector.memset(ones_f[:], 1.0)
        neg20 = const.tile([128, 1], F32, tag="neg20")
        nc.vector.memset(neg20[:], -SMOOTH)
        ones_row = const.tile([1, DH], F32, tag="ones_row")
        nc.vector.memset(ones_row[:], 1.0)
        ident = const.tile([128, 128], F16, tag="ident")

        # ---- resident loads (priority-ordered) ----
        law4 = const.tile([128, NST * 4 * TQ], F16, tag="law4")
        qT = const.tile([DH, H * TQ], F16, tag="qT")
        kTe = big.tile([DH, H * S], F16, tag="kTe")
        bias_pre = []
        with tc.high_priority():
            # the very first PE op is the bias preload: ident + bias(0) must
            # head the queues, ahead of even kTe/qT
            nc.sync.dma_start(out=ident[:], in_=d_id)
            for pit in range(2):
                bt = biasp.tile([128, 4 * TQ], F16, tag="bias")
                beng = nc.sync if pit % 2 == 0 else nc.scalar
                beng.dma_start(out=bt[:],
                               in_=d_bias[pit * 128:(pit + 1) * 128, :])
                bias_pre.append(bt)
            nc.sync.dma_start(out=kTe[:, :4 * S], in_=d_kTe[:, :4 * S])
            nc.sync.dma_start(out=qT[:, :4 * TQ], in_=d_qT[:, :4 * TQ])
            nc.scalar.dma_start(out=law4[:, :2048], in_=d_law4[:, :2048])
            for pit in range(2, 4):
                bt = biasp.tile([128, 4 * TQ], F16, tag="bias")
                beng = nc.sync if pit % 2 == 0 else nc.scalar
                beng.dma_start(out=bt[:],
                               in_=d_bias[pit * 128:(pit + 1) * 128, :])
                bias_pre.append(bt)
        nc.scalar.dma_start(out=law4[:, 2048:], in_=d_law4[:, 2048:])

        # V tiles: 4 direct + 2 gathered (PBC expansion), f16
        v_sb = []
        for st in range(4):
            vt = const.tile([128, P * HID + 128], F16, tag=f"v{st}",
                            name=f"v{st}")
            nc.vector.memset(vt[:, P * HID:], 0.0)
            v_sb.append(vt)
        nc.gpsimd.dma_start(out=v_sb[0][:, :P * HID], in_=d_vb[0:128, :])
        nc.gpsimd.dma_start(out=v_sb[1][:, :P * HID], in_=d_vb[128:256, :])
        idx_sb = const.tile([128, 2], I32, tag="idx")
        nc.gpsimd.dma_start(
            out=idx_sb[:].rearrange("p (two one) -> p two one", one=1),
            in_=d_vidx.rearrange("two p one -> p two one"))
        for gi in range(2):
            vt = const.tile([128, P * HID + 128], F16, tag=f"v{4 + gi}",
                            name=f"vg{gi}")
            nc.vector.memset(vt[:, P * HID:], 0.0)
            nc.gpsimd.indirect_dma_start(
                out=vt[:, :P * HID], out_offset=None,
                in_=d_vb[:, :],
                in_offset=bass.IndirectOffsetOnAxis(
                    ap=idx_sb[:, gi:gi + 1], axis=0))
            v_sb.append(vt)
        nc.scalar.dma_start(out=v_sb[2][:, :P * HID], in_=d_vb[256:384, :])
        nc.scalar.dma_start(out=v_sb[3][:, :P * HID], in_=d_vb[384:512, :])
        nc.sync.dma_start(out=kTe[:, 4 * S:8 * S], in_=d_kTe[:, 4 * S:8 * S])
        nc.sync.dma_start(out=qT[:, 4 * TQ:8 * TQ],
                          in_=d_qT[:, 4 * TQ:8 * TQ])
        for c in range(2, 4):
            nc.sync.dma_start(
                out=kTe[:, c * 4 * S:(c + 1) * 4 * S],
                in_=d_kTe[:, c * 4 * S:(c + 1) * 4 * S])
            nc.sync.dma_start(
                out=qT[:, c * 4 * TQ:(c + 1) * 4 * TQ],
                in_=d_qT[:, c * 4 * TQ:(c + 1) * 4 * TQ])
        wT = const.tile([128, 4 * HID], F16, tag="wT")          # (c%128,(ci,o))
        nc.scalar.dma_start(out=wT[:], in_=d_wT)

        # attn channel-major tiles: [128 (c%128), (p, t)], c = h*32+dd
        attn_ct = []
        for ci in range(4):
            attn_ct.append(attnp.tile([128, P * TQ], F16, tag=f"act{ci}",
                                      name=f"act{ci}"))

        # PSUM accumulator that lives across the whole run. Zeroed once and
        # accumulated with start=False throughout: its two single-column
        # accumulation regions share one bank, and a second region's
        # start=True would reset the first region's partial.
        ssq_col = psum1.tile([128, 2], F32, space="PSUM", tag="ssq")
        nc.vector.memset(ssq_col[:], 0.0)

        # ============ software-pipelined attention main loop ============
        numer = None
        den2 = None
        numer_pending = den_pending = None
        prev = None            # (e, g) of iteration it-1
        drain2_pending = None
        remap_engines = [nc.sync, nc.gpsimd, nc.scalar]

        for it in range(NIT + 2):
            if it < NIT:
                hg, st = divmod(it, NST)
                if st == 0:
                    numer_new = [psum1.tile([128, 2 * TQ], F32, space="PSUM",
                                            tag=f"numer_{i}",
                                            name=f"numer{hg}_{i}")
                                 for i in range(2)]
                    # pair j at partition j*32 (matmul base-partition rule)
                    den2_new = psum1.tile([33, 2 * TQ], F32, space="PSUM",
                                          tag="den", name=f"den{hg}")
                if it < 4:
                    bt = bias_pre[it]
                else:
                    bt = biasp.tile([128, 4 * TQ], F16, tag="bias")
                    nc.sync.dma_start(out=bt[:],
                                      in_=d_bias[it * 128:(it + 1) * 128, :])
                scores = psum.tile([128, 4 * TQ], F32, space="PSUM",
                                   tag="scores")
                # bias preload (identity matmul, f16 full rate)
                for half in range(2):
                    nc.tensor.matmul(
                        out=scores[:, half * 512:(half + 1) * 512],
                        lhsT=ident[:],
                        rhs=bt[:, half * 512:(half + 1) * 512],
                        start=True, stop=False)
                # QK accumulate on top (i=1,3 close their banks)
                for i in range(4):
                    h = hg * 4 + i
                    nc.tensor.matmul(
                        out=scores[:, i * TQ:(i + 1) * TQ],
                        lhsT=kTe[:, h * S + st * 128:h * S + (st + 1) * 128],
                        rhs=qT[:, h * TQ:(h + 1) * TQ],
                        start=False, stop=(i % 2 == 1))
                law_flat = law4[:, st * 1024:(st + 1) * 1024]
                # m = (scores + 20) * law    [DVE, f32; only PSUM reader]
                m = work.tile([128, 4 * TQ], F32, tag="m")
                nc.vector.scalar_tensor_tensor(
                    out=m[:], in0=scores[:],
                    scalar=SMOOTH, in1=law_flat, op0=ALU.add, op1=ALU.mult)
                # e = exp(m - 20) -> f16     [ACT]
                e = work.tile([128, 4 * TQ], F16, tag="e")
                nc.scalar.activation(e[:], m[:], AF.Exp, bias=neg20[:],
                                     scale=1.0)
                # g = e * law4 (flat) [f16] - split DVE/Pool within the
                # iteration so neither engine's latency gates numer alone
                g = work.tile([128, 4 * TQ], F16, tag="g")
                nc.vector.tensor_tensor(
                    out=g[:, :640], in0=e[:, :640],
                    in1=law_flat[:, :640], op=ALU.mult)
                nc.gpsimd.tensor_tensor(
                    out=g[:, 640:], in0=e[:, 640:],
                    in1=law_flat[:, 640:], op=ALU.mult)
                if d_dbg is not None and hg == 0 and st == 0:
                    nc.sync.dma_start(out=d_dbg["e"], in_=e[:])
                    nc.sync.dma_start(out=d_dbg["g"], in_=g[:])
                if st == 0:
                    numer_pending, den_pending = numer_new, den2_new
            if 1 <= it <= NIT:
                phg, pst = divmod(it - 1, NST)
                if pst == 0:
                    numer, den2 = numer_pending, den_pending
                pe, pg = prev
                # denominators: ones.T @ e -> den2[j*32, (jj, t)]
                for j in range(2):
                    nc.tensor.matmul(
                        out=den2[j * 32:j * 32 + 1, :],
                        lhsT=ones_h[:, 0:1],
                        rhs=pe[:, j * 512:(j + 1) * 512],
                        start=(pst == 0), stop=(pst == NST - 1))
                # numerators: v.T @ g -> [96(+32), t] per head
                for i in range(4):
                    h = phg * 4 + i
                    nc.tensor.matmul(
                        out=numer[i // 2][:, (i % 2) * TQ:(i % 2 + 1) * TQ],
                        lhsT=v_sb[pst][:, h * DH:h * DH + 128],
                        rhs=pg[:, i * TQ:(i + 1) * TQ],
                        start=(pst == 0 and i % 2 == 0),
                        stop=(pst == NST - 1 and i % 2 == 1))
                if pst == NST - 1:
                    # ---- drain stage 1 for head-group phg ----
                    # pair 1's den sits at PSUM partition 32; custom DVE ops
                    # misread non-zero base partitions -> bounce via ACT copy
                    d1row = drain.tile([1, 2 * TQ], F32, tag="d1row")
                    nc.scalar.copy(d1row[:], den2[32:33, :])
                    adivs = []
                    for i in range(2):
                        rc = drain.tile([1, 2 * TQ], F32, tag="rec2")
                        nc.vector.reciprocal_approx_fast(
                            out=rc[:],
                            in_=(den2[0:1, :] if i == 0 else d1row[:]))
                        adiv = drain.tile([96, 2 * TQ], F16, tag="adiv",
                                          name=f"adiv{phg}_{i}")
                        if phg == HG - 1:
                            # last hg is tail-critical: broadcast rec via a
                            # rank-1 PE matmul (scores banks are free now);
                            # DVE divide may read only ONE PSUM operand, so
                            # numer still bounces through an ACT f16 copy.
                            rb_ps = psum.tile([DH, 2 * TQ], F32, space="PSUM",
                                              tag="scores")
                            nc.tensor.matmul(
                                out=rb_ps[:, :], lhsT=ones_row[:],
                                rhs=rc[:], start=True, stop=True)
                            anum = drain.tile([96, 2 * TQ], F16, tag="anum")
                            nc.scalar.copy(anum[:], numer[i][:96, :])
                            nc.vector.tensor_tensor(
                                out=adiv[:], in0=anum[:],
                                in1=rb_ps[:96, :], op=ALU.mult)
                        else:
                            # broadcast via DRAM bounce (partition_broadcast
                            # on Pool forces a gpsimd library swap ~11us)
                            nc.sync.dma_start(
                                out=d_rec[phg * 2 + i:phg * 2 + i + 1, :],
                                in_=rc[:])
                            rb = drain.tile([96, 2 * TQ], F32, tag="rb")
                            nc.gpsimd.dma_start(
                                out=rb[:],
                                in_=d_rec[phg * 2 + i:phg * 2 + i + 1, :]
                                    .to_broadcast((96, 2 * TQ)))
                            # numer PSUM -> f16 SBUF (Pool can't read PSUM)
                            anum = drain.tile([96, 2 * TQ], F16, tag="anum")
                            nc.scalar.copy(anum[:], numer[i][:96, :])
                            nc.gpsimd.tensor_tensor(
                                out=adiv[:], in0=anum[:], in1=rb[:],
                                op=ALU.mult)
                        adivs.append(adiv)
                    drain2_pending = (phg, adivs)
            if it >= 2 and (it - 2) % NST == NST - 1 and drain2_pending:
                # ---- drain stage 2 (one block later: keeps the PE's ssq
                # matmuls from stalling the stream on the ACT/Pool chain) ----
                phg2, adivs2 = drain2_pending
                drain2_pending = None
                for i in range(2):
                    adiv = adivs2[i]
                    sq = drain.tile([96, 2 * TQ], F16, tag="sq")
                    if phg2 == HG - 1:
                        nc.vector.tensor_tensor(out=sq[:], in0=adiv[:],
                                                in1=adiv[:], op=ALU.mult)
                    else:
                        nc.scalar.square(sq[:], adiv[:])
                    # transposed ssq: [t%128, 1] += ones over 96 channels
                    for j in range(2):
                        for thh in range(2):
                            nc.tensor.matmul(
                                out=ssq_col[:, thh:thh + 1],
                                lhsT=sq[:, j * TQ + thh * 128:
                                        j * TQ + (thh + 1) * 128],
                                rhs=ones_h[0:96, 0:1],
                                start=False,
                                stop=(phg2 == HG - 1 and i == 1 and j == 1))
                    # remap (h, p, dd) rows -> channel-major attn_ct[phg2]
                    for j in range(2):
                        for p in range(P):
                            eng = remap_engines[(i * 6 + j * 3 + p) % 3]
                            eng.dma_start(
                                out=attn_ct[phg2][
                                    (2 * i + j) * 32:(2 * i + j + 1) * 32,
                                    p * TQ:(p + 1) * TQ],
                                in_=adiv[p * 32:(p + 1) * 32,
                                         j * TQ:(j + 1) * TQ])
            if it < NIT:
                prev = (e, g)

        # ================= equivariant LN (column form) =================
        # inorm = rsqrt(ssq/512 + eps), one Newton step; icol[t%128, th]
        acol = const.tile([128, 2], F32, tag="acol")
        nc.vector.tensor_scalar(
            out=acol[:], in0=ssq_col[:, :], scalar1=1.0 / HID, scalar2=EPS,
            op0=ALU.mult, op1=ALU.add)
        rcpa = const.tile([128, 2], F32, tag="rcpa")
        nc.vector.reciprocal_approx_fast(out=rcpa[:], in_=acol[:])
        r0t = const.tile([128, 2], F32, tag="r0t")
        nc.scalar.activation(r0t[:], rcpa[:], AF.Sqrt, bias=0.0, scale=1.0)
        tmp = const.tile([128, 2], F32, tag="tmpn")
        nc.vector.tensor_tensor(out=tmp[:], in0=r0t[:], in1=r0t[:],
                                op=ALU.mult)
        nc.vector.tensor_tensor(out=tmp[:], in0=tmp[:], in1=acol[:],
                                op=ALU.mult)
        nc.vector.tensor_scalar(
            out=tmp[:], in0=tmp[:], scalar1=-0.5, scalar2=1.5,
            op0=ALU.mult, op1=ALU.add)
        icol = const.tile([128, 2], F32, tag="icol")
        nc.vector.tensor_tensor(out=icol[:], in0=r0t[:], in1=tmp[:],
                                op=ALU.mult)
        if d_dbg is not None:
            for ci in range(4):
                nc.sync.dma_start(
                    out=d_dbg["attn"][ci * 128:(ci + 1) * 128, :],
                    in_=attn_ct[ci][:])

        # ================= out_proj =================
        for k in range(6):          # tiles: p = k // 2, t-half = k % 2
            p, th = k // 2, k % 2
            op = psum.tile([128, HID], F32, space="PSUM", tag="scores")
            for ci in range(4):
                nc.tensor.matmul(
                    out=op[:, :],
                    lhsT=attn_ct[ci][:, p * TQ + th * 128:
                                     p * TQ + (th + 1) * 128],
                    rhs=wT[:, ci * HID:(ci + 1) * HID],
                    start=(ci == 0), stop=(ci == 3))
            ot = drain.tile([128, HID], F32, tag="osb")
            if k % 2 == 0:
                nc.scalar.activation(ot[:], op[:, :], AF.Copy, bias=0.0,
                                     scale=icol[:, th:th + 1])
            else:
                nc.vector.tensor_scalar(
                    out=ot[:], in0=op[:, :], scalar1=icol[:, th:th + 1],
                    scalar2=None, op0=ALU.mult)
            remap_engines[k % 3].dma_start(
                out=d_out[th * 128:th * 128 + 128, p, :],
                in_=ot[:])


def _host_prep(q, k, v, attn_bias, local_attention_weight, out_proj_w,
               ln_weight, outcell_index):
    """Pure layout marshalling on host -> per-core input dicts."""
    q = np.asarray(q, np.float32)
    k = np.asarray(k, np.float32)
    v = np.asarray(v, np.float32)
    attn_bias = np.asarray(attn_bias, np.float32)
    law = np.asarray(local_attention_weight, np.float32)
    out_proj_w = np.asarray(out_proj_w, np.float32)
    ln_weight = np.asarray(ln_weight, np.float32)
    idx = np.asarray(outcell_index).astype(np.int64)

    # (B,T,P,HID) -> (B, 96, H, T) with row j = p*32+dd
    def to_dT(x):
        return np.ascontiguousarray(
            x.reshape(B, T, P, H, D).transpose(0, 2, 4, 3, 1)
        ).reshape(B, P * D, H, T)

    qT = to_dT(q) * np.float32(SCALING)
    kT = to_dT(k)
    kTe = np.concatenate(
        [kT, np.take_along_axis(
            kT, idx[:, None, None, :].astype(np.int64), axis=3)], axis=3)
    lawT = np.ascontiguousarray(law.transpose(0, 2, 1))            # (B,S,T)
    biasT = np.ascontiguousarray(
        attn_bias.transpose(0, 3, 1, 2)).astype(np.float16)        # (B,S,H,T)
    vb = np.ascontiguousarray(
        v.reshape(B, T, P, H, D).transpose(0, 1, 3, 2, 4)
    ).reshape(B, T, P * HID).astype(np.float16)
    wln = (np.ascontiguousarray(out_proj_w.T) * ln_weight[:, None]
           ).astype(np.float32)
    wTimg = np.ascontiguousarray(
        wln.reshape(4, 128, HID).transpose(1, 0, 2)
    ).reshape(128, 4 * HID).astype(np.float16)
    vidx = idx.astype(np.int32).reshape(B, 2, 128, 1)

    in_maps = []
    for c in range(NCORES):
        b, th = c // 2, c % 2
        t0 = th * TQ
        lawc = np.ascontiguousarray(lawT[b, :, t0:t0 + TQ])
        lawc = np.ascontiguousarray(
            lawc.reshape(NST, 128, TQ).transpose(1, 0, 2)
        ).reshape(128, NST * TQ).astype(np.float16)
        # law4: per-st tile with the t-block replicated for the 4 heads
        law4c = np.ascontiguousarray(
            np.broadcast_to(
                lawc.reshape(128, NST, 1, TQ), (128, NST, 4, TQ))
        ).reshape(128, NST * 4 * TQ)
        # bias image: [(hg*NST+st)*128 + s%128, (i, t)]
        bc = biasT[b, :, :, t0:t0 + TQ]            # (S, H, TQ)
        bc = np.ascontiguousarray(
            bc.reshape(NST, 128, HG, 4, TQ).transpose(2, 0, 1, 3, 4)
        ).reshape(NIT * 128, 4 * TQ)
        in_maps.append(dict(
            biasT=bc,
            law4=law4c,
            qT=np.ascontiguousarray(
                qT[b, :, :, t0:t0 + TQ]).reshape(DH, H * TQ)
                .astype(np.float16),
            kTe=np.ascontiguousarray(kTe[b]).reshape(DH, H * S)
                .astype(np.float16),
            vb=np.ascontiguousarray(vb[b]),
            vidx=np.ascontiguousarray(vidx[b]),
            wT=wTimg,
            ident=np.eye(128, dtype=np.float16),
        ))
    return in_maps


def kernel(**inputs):
    global _CACHED_NC
    if _CACHED_NC is None:
        _CACHED_NC = build_nc()
    nc = _CACHED_NC
    in_maps = _host_prep(
        inputs["q"], inputs["k"], inputs["v"], inputs["attn_bias"],
        inputs["local_attention_weight"], inputs["out_proj_w"],
        inputs["ln_weight"], inputs["outcell_index"])
    res = run_bass_kernel_spmd(nc, in_maps, core_ids=list(range(NCORES)))
    out = np.empty((B, T, P, HID), np.float32)
    for c in range(NCORES):
        b, th = c // 2, c % 2
        out[b, th * TQ:(th + 1) * TQ] = res.results[c]["out"]
    return out


# revision 62
# speedup vs baseline: 1.0296x; 1.0296x over previous
"""Trainium2 Bass kernel for EquivariantAttention (sparse_attention).

Full (unsharded) inputs in, full output out. Internally shards over the 8
NeuronCores as (batch, T-half): core c handles batch b = c // 2, query rows
t0 = (c % 2) * 256 .. t0+256.  Every core runs the identical SPMD program on
its own input slices; there is no cross-core communication (LN and out_proj
are row-local in (b, t)).

v5 design (all-f16 datapath, software-pipelined):
  Per (head-group hg of 4 heads, s-tile st of 128):
    scores_T[s,t] = bias_T + k_T.T @ q_T     (bias preloaded into PSUM via
                                              identity matmul @ f16 full rate,
                                              QK accumulates on top)
    m = (scores_T + 20) * law                (DVE stt, f32 - only PSUM reader)
    e = exp(m - 20) -> f16                   (ACT, per-partition bias)
    den[t] += sum_s e                        (ones-column matmul)
    g = e * law4 (flat, no broadcast AP)     (f16; alternate DVE/Pool)
    numer[j,t] += sum_s v[s,j] * g[s,t]      (f16 matmul; PBC rows of V
                                              gathered on device)
  The loop is software-pipelined one deep: preload/QK/elementwise of iter n+1
  issue before den/numer of iter n so the PE never waits on the DVE/ACT chain.
  Per-hg drain (overlapped with the next hg's loop):
    rec = recip(den)  (pair 1 bounced via ACT copy: custom DVE ops misread
                       PSUM at partition offset 32)
    rb = partition_broadcast(rec);  anum = ACT copy of numer -> f16
    adiv = anum*rb (Pool);  sq = ACT square;  transposed ssq ones-matmuls
    remap adiv -> channel-major attn_ct[hg] via 12 small SBUF DMAs
  Tail: inorm = rsqrt(ssq/512+eps) directly in column form; out_proj f16
  matmuls with inorm folded into the PSUM->SBUF ACT copy.
"""

import numpy as np

import concourse.bass as bass
import concourse.bacc as bacc
import concourse.tile as tile
from concourse import mybir
from concourse.bass_utils import run_bass_kernel_spmd

# Problem constants (hardcoded per contract)
B, T, P, HID = 4, 512, 3, 512
H, D = 16, 32
EXP = 256
S = T + EXP            # 768
SCALING = (D / 3.0) ** 0.5 / D
SMOOTH = 20.0
EPS = 1e-3

NCORES = 8
TQ = T // 2            # 256 query rows per core
DH = P * D             # 96 head dim
NST = S // 128         # 6 s-tiles of 128
HG = 4                 # head groups of 4 heads
NIT = HG * NST         # 24 pipelined iterations

F32 = mybir.dt.float32
F16 = mybir.dt.float16
I32 = mybir.dt.int32
AF = mybir.ActivationFunctionType
ALU = mybir.AluOpType

_CACHED_NC = None
DEBUG_DUMPS = False


def build_nc():
    nc = bacc.Bacc("TRN2", target_bir_lowering=False, debug=False)

    # ---- DRAM I/O (per-core shapes) ----
    d_bias = nc.dram_tensor("biasT", [NIT * 128, 4 * TQ], F16,
                            kind="ExternalInput").ap()
    d_law4 = nc.dram_tensor("law4", [128, NST * 4 * TQ], F16,
                            kind="ExternalInput").ap()
    d_rec = nc.dram_tensor("rec_scratch", [8, 2 * TQ], F32).ap()
    d_qT = nc.dram_tensor("qT", [DH, H * TQ], F16, kind="ExternalInput").ap()
    d_kTe = nc.dram_tensor("kTe", [DH, H * S], F16, kind="ExternalInput").ap()
    d_vb = nc.dram_tensor("vb", [T, P * HID], F16, kind="ExternalInput").ap()
    d_vidx = nc.dram_tensor("vidx", [2, 128, 1], I32, kind="ExternalInput").ap()
    d_wT = nc.dram_tensor("wT", [128, 4 * HID], F16, kind="ExternalInput").ap()
    d_id = nc.dram_tensor("ident", [128, 128], F16, kind="ExternalInput").ap()
    d_out = nc.dram_tensor("out", [TQ, P, HID], F32, kind="ExternalOutput").ap()
    d_dbg = None
    if DEBUG_DUMPS:
        d_dbg = dict(
            attn=nc.dram_tensor("dbg_attn", [4 * 128, P * TQ], F16,
                                kind="ExternalOutput").ap(),
            e=nc.dram_tensor("dbg_e", [128, 4 * TQ], F16,
                             kind="ExternalOutput").ap(),
            g=nc.dram_tensor("dbg_g", [128, 4 * TQ], F16,
                             kind="ExternalOutput").ap(),
        )

    with tile.TileContext(nc) as tc:
        build_kernel(tc, d_bias, d_law4, d_qT, d_kTe, d_vb, d_vidx,
                     d_wT, d_id, d_out, d_rec, d_dbg)
    nc.compile()
    return nc


def build_kernel(tc, d_bias, d_law4, d_qT, d_kTe, d_vb, d_vidx,
                 d_wT, d_id, d_out, d_rec, d_dbg=None):
    nc = tc.nc
    from contextlib import ExitStack, nullcontext
    ctx = ExitStack()
    with ctx:
        const = ctx.enter_context(tc.tile_pool(name="const", bufs=1))
        big = ctx.enter_context(tc.tile_pool(name="big", bufs=1))
        biasp = ctx.enter_context(tc.tile_pool(name="biasp", bufs=6))
        work = ctx.enter_context(tc.tile_pool(name="work", bufs=4))
        drain = ctx.enter_context(tc.tile_pool(name="drain", bufs=2))
        attnp = ctx.enter_context(tc.tile_pool(name="attnp", bufs=1))
        psum = ctx.enter_context(tc.tile_pool(name="psum", bufs=2, space="PSUM"))
        psum1 = ctx.enter_context(tc.tile_pool(name="psum1", bufs=1, space="PSUM"))

        # ---- constants ----
        ones_h = const.tile([128, 1], F16, tag="ones_h")
        nc.vector.memset(ones_h[:], 1.0)
        # touch the Pool engine once so its tensor-op firmware library loads
        # during DMA warmup instead of stalling the first g-multiply (~5.6us)
        poolwarm = const.tile([128, 1], F16, tag="poolwarm")
        nc.gpsimd.tensor_tensor(out=poolwarm[:], in0=ones_h[:],
                                in1=ones_h[:], op=ALU.mult)
        ones_f = const.tile([128, 1], F32, tag="ones_f")
        nc.vector.memset(ones_f[:], 1.0)
        neg20 = const.tile([128, 1], F32, tag="neg20")
        nc.vector.memset(neg20[:], -SMOOTH)
        ones_row = const.tile([1, DH], F32, tag="ones_row")
        nc.vector.memset(ones_row[:], 1.0)
        ident = const.tile([128, 128], F16, tag="ident")
        nc.sync.dma_start(out=ident[:], in_=d_id)

        # ---- resident loads (priority-ordered) ----
        law4 = const.tile([128, NST * 4 * TQ], F16, tag="law4")
        qT = const.tile([DH, H * TQ], F16, tag="qT")
        kTe = big.tile([DH, H * S], F16, tag="kTe")
        bias_pre = []
        with tc.high_priority():
            nc.sync.dma_start(out=kTe[:, :4 * S], in_=d_kTe[:, :4 * S])
            nc.sync.dma_start(out=qT[:, :4 * TQ], in_=d_qT[:, :4 * TQ])
            nc.scalar.dma_start(out=law4[:, :2048], in_=d_law4[:, :2048])
            # first bias tiles gate the first PE preload - fetch them at the
            # head of the sync/scalar queues
            for pit in range(4):
                bt = biasp.tile([128, 4 * TQ], F16, tag="bias")
                beng = nc.sync if pit % 2 == 0 else nc.scalar
                beng.dma_start(out=bt[:],
                               in_=d_bias[pit * 128:(pit + 1) * 128, :])
                bias_pre.append(bt)
        nc.scalar.dma_start(out=law4[:, 2048:], in_=d_law4[:, 2048:])

        # V tiles: 4 direct + 2 gathered (PBC expansion), f16
        v_sb = []
        for st in range(4):
            vt = const.tile([128, P * HID + 128], F16, tag=f"v{st}",
                            name=f"v{st}")
            nc.vector.memset(vt[:, P * HID:], 0.0)
            v_sb.append(vt)
        nc.gpsimd.dma_start(out=v_sb[0][:, :P * HID], in_=d_vb[0:128, :])
        nc.gpsimd.dma_start(out=v_sb[1][:, :P * HID], in_=d_vb[128:256, :])
        idx_sb = const.tile([128, 2], I32, tag="idx")
        nc.gpsimd.dma_start(
            out=idx_sb[:].rearrange("p (two one) -> p two one", one=1),
            in_=d_vidx.rearrange("two p one -> p two one"))
        for gi in range(2):
            vt = const.tile([128, P * HID + 128], F16, tag=f"v{4 + gi}",
                            name=f"vg{gi}")
            nc.vector.memset(vt[:, P * HID:], 0.0)
            nc.gpsimd.indirect_dma_start(
                out=vt[:, :P * HID], out_offset=None,
                in_=d_vb[:, :],
                in_offset=bass.IndirectOffsetOnAxis(
                    ap=idx_sb[:, gi:gi + 1], axis=0))
            v_sb.append(vt)
        nc.scalar.dma_start(out=v_sb[2][:, :P * HID], in_=d_vb[256:384, :])
        nc.scalar.dma_start(out=v_sb[3][:, :P * HID], in_=d_vb[384:512, :])
        nc.sync.dma_start(out=kTe[:, 4 * S:8 * S], in_=d_kTe[:, 4 * S:8 * S])
        nc.sync.dma_start(out=qT[:, 4 * TQ:8 * TQ],
                          in_=d_qT[:, 4 * TQ:8 * TQ])
        for c in range(2, 4):
            nc.sync.dma_start(
                out=kTe[:, c * 4 * S:(c + 1) * 4 * S],
                in_=d_kTe[:, c * 4 * S:(c + 1) * 4 * S])
            nc.sync.dma_start(
                out=qT[:, c * 4 * TQ:(c + 1) * 4 * TQ],
                in_=d_qT[:, c * 4 * TQ:(c + 1) * 4 * TQ])
        wT = const.tile([128, 4 * HID], F16, tag="wT")          # (c%128,(ci,o))
        nc.scalar.dma_start(out=wT[:], in_=d_wT)

        # attn channel-major tiles: [128 (c%128), (p, t)], c = h*32+dd
        attn_ct = []
        for ci in range(4):
            attn_ct.append(attnp.tile([128, P * TQ], F16, tag=f"act{ci}",
                                      name=f"act{ci}"))

        # PSUM accumulator that lives across the whole run. Zeroed once and
        # accumulated with start=False throughout: its two single-column
        # accumulation regions share one bank, and a second region's
        # start=True would reset the first region's partial.
        ssq_col = psum1.tile([128, 2], F32, space="PSUM", tag="ssq")
        nc.vector.memset(ssq_col[:], 0.0)

        # ============ software-pipelined attention main loop ============
        numer = None
        den2 = None
        numer_pending = den_pending = None
        prev1 = None           # (e, g) of iteration it-1
        prev2 = None           # (e, g) of iteration it-2
        den_by_hg = {}
        numer_by_hg = {}
        drain2_pending = None
        remap_engines = [nc.sync, nc.gpsimd, nc.scalar]

        for it in range(NIT + 3):
            if it < NIT:
                hg, st = divmod(it, NST)
                if st == 0:
                    numer_new = [psum1.tile([128, 2 * TQ], F32, space="PSUM",
                                            tag=f"numer_{i}",
                                            name=f"numer{hg}_{i}")
                                 for i in range(2)]
                    # pair j at partition j*32 (matmul base-partition rule)
                    den2_new = psum1.tile([33, 2 * TQ], F32, space="PSUM",
                                          tag="den", name=f"den{hg}")
                if it < 4:
                    bt = bias_pre[it]
                else:
                    bt = biasp.tile([128, 4 * TQ], F16, tag="bias")
                    nc.sync.dma_start(out=bt[:],
                                      in_=d_bias[it * 128:(it + 1) * 128, :])
                scores = psum.tile([128, 4 * TQ], F32, space="PSUM",
                                   tag="scores")
                # bias preload (identity matmul, f16 full rate)
                for half in range(2):
                    nc.tensor.matmul(
                        out=scores[:, half * 512:(half + 1) * 512],
                        lhsT=ident[:],
                        rhs=bt[:, half * 512:(half + 1) * 512],
                        start=True, stop=False)
                # QK accumulate on top (i=1,3 close their banks)
                for i in range(4):
                    h = hg * 4 + i
                    nc.tensor.matmul(
                        out=scores[:, i * TQ:(i + 1) * TQ],
                        lhsT=kTe[:, h * S + st * 128:h * S + (st + 1) * 128],
                        rhs=qT[:, h * TQ:(h + 1) * TQ],
                        start=False, stop=(i % 2 == 1))
                law_flat = law4[:, st * 1024:(st + 1) * 1024]
                # m = (scores + 20) * law    [DVE, f32; only PSUM reader]
                m = work.tile([128, 4 * TQ], F32, tag="m")
                nc.vector.scalar_tensor_tensor(
                    out=m[:], in0=scores[:],
                    scalar=SMOOTH, in1=law_flat, op0=ALU.add, op1=ALU.mult)
                # e = exp(m - 20) -> f16     [ACT]
                e = work.tile([128, 4 * TQ], F16, tag="e")
                nc.scalar.activation(e[:], m[:], AF.Exp, bias=neg20[:],
                                     scale=1.0)
                # g = e * law4 (flat)        [f16; alternate DVE/Pool]
                g = work.tile([128, 4 * TQ], F16, tag="g")
                geng = nc.vector if st % 2 == 0 else nc.gpsimd
                geng.tensor_tensor(
                    out=g[:], in0=e[:], in1=law_flat, op=ALU.mult)
                if d_dbg is not None and hg == 0 and st == 0:
                    nc.sync.dma_start(out=d_dbg["e"], in_=e[:])
                    nc.sync.dma_start(out=d_dbg["g"], in_=g[:])
                if st == 0:
                    numer_pending, den_pending = numer_new, den2_new
                    den_by_hg[hg] = den2_new
                    numer_by_hg[hg] = numer_new
            # numer runs TWO iterations behind (den only one): the stt->exp->g
            # chain takes ~3.3us while preload+QK+den give only ~1.7us of
            # cover, so a 1-deep numer stalled the PE ~1.6us per iteration.
            if 2 <= it <= NIT + 1:
                phg, pst = divmod(it - 2, NST)
                numer = numer_by_hg[phg]
                pg = prev2[1]
                # numerators: v.T @ g -> [96(+32), t] per head
                for i in range(4):
                    h = phg * 4 + i
                    nc.tensor.matmul(
                        out=numer[i // 2][:, (i % 2) * TQ:(i % 2 + 1) * TQ],
                        lhsT=v_sb[pst][:, h * DH:h * DH + 128],
                        rhs=pg[:, i * TQ:(i + 1) * TQ],
                        start=(pst == 0 and i % 2 == 0),
                        stop=(pst == NST - 1 and i % 2 == 1))
                den2 = den_by_hg[phg]
                if pst == NST - 1:
                    # ---- drain stage 1 for head-group phg ----
                    # pair 1's den sits at PSUM partition 32; custom DVE ops
                    # misread non-zero base partitions -> bounce via ACT copy
                    d1row = drain.tile([1, 2 * TQ], F32, tag="d1row")
                    nc.scalar.copy(d1row[:], den2[32:33, :])
                    adivs = []
                    for i in range(2):
                        rc = drain.tile([1, 2 * TQ], F32, tag="rec2")
                        nc.vector.reciprocal_approx_fast(
                            out=rc[:],
                            in_=(den2[0:1, :] if i == 0 else d1row[:]))
                        adiv = drain.tile([96, 2 * TQ], F16, tag="adiv",
                                          name=f"adiv{phg}_{i}")
                        if phg == HG - 1:
                            # last hg is tail-critical: broadcast rec via a
                            # rank-1 PE matmul (scores banks are free now);
                            # DVE divide may read only ONE PSUM operand, so
                            # numer still bounces through an ACT f16 copy.
                            rb_ps = psum.tile([DH, 2 * TQ], F32, space="PSUM",
                                              tag="scores")
                            nc.tensor.matmul(
                                out=rb_ps[:, :], lhsT=ones_row[:],
                                rhs=rc[:], start=True, stop=True)
                            anum = drain.tile([96, 2 * TQ], F16, tag="anum")
                            nc.scalar.copy(anum[:], numer[i][:96, :])
                            nc.vector.tensor_tensor(
                                out=adiv[:], in0=anum[:],
                                in1=rb_ps[:96, :], op=ALU.mult)
                        else:
                            # broadcast via DRAM bounce (partition_broadcast
                            # on Pool forces a gpsimd library swap ~11us)
                            nc.sync.dma_start(
                                out=d_rec[phg * 2 + i:phg * 2 + i + 1, :],
                                in_=rc[:])
                            rb = drain.tile([96, 2 * TQ], F32, tag="rb")
                            nc.gpsimd.dma_start(
                                out=rb[:],
                                in_=d_rec[phg * 2 + i:phg * 2 + i + 1, :]
                                    .to_broadcast((96, 2 * TQ)))
                            # numer PSUM -> f16 SBUF (Pool can't read PSUM)
                            anum = drain.tile([96, 2 * TQ], F16, tag="anum")
                            nc.scalar.copy(anum[:], numer[i][:96, :])
                            nc.gpsimd.tensor_tensor(
                                out=adiv[:], in0=anum[:], in1=rb[:],
                                op=ALU.mult)
                        adivs.append(adiv)
                    drain2_pending = (phg, adivs)
            if 1 <= it <= NIT:
                dhg, dst = divmod(it - 1, NST)
                dden = den_by_hg[dhg]
                pe = prev1[0]
                # denominators: ones.T @ e -> den[j*32, (jj, t)]
                for j in range(2):
                    nc.tensor.matmul(
                        out=dden[j * 32:j * 32 + 1, :],
                        lhsT=ones_h[:, 0:1],
                        rhs=pe[:, j * 512:(j + 1) * 512],
                        start=(dst == 0), stop=(dst == NST - 1))
            if it >= 3 and (it - 3) % NST == NST - 1 and drain2_pending:
                # ---- drain stage 2 (one block later: keeps the PE's ssq
                # matmuls from stalling the stream on the ACT/Pool chain) ----
                phg2, adivs2 = drain2_pending
                drain2_pending = None
                for i in range(2):
                    adiv = adivs2[i]
                    sq = drain.tile([96, 2 * TQ], F16, tag="sq")
                    if phg2 == HG - 1:
                        nc.vector.tensor_tensor(out=sq[:], in0=adiv[:],
                                                in1=adiv[:], op=ALU.mult)
                    else:
                        nc.scalar.square(sq[:], adiv[:])
                    # transposed ssq: [t%128, 1] += ones over 96 channels
                    for j in range(2):
                        for thh in range(2):
                            nc.tensor.matmul(
                                out=ssq_col[:, thh:thh + 1],
                                lhsT=sq[:, j * TQ + thh * 128:
                                        j * TQ + (thh + 1) * 128],
                                rhs=ones_h[0:96, 0:1],
                                start=False,
                                stop=(phg2 == HG - 1 and i == 1 and j == 1))
                    # remap (h, p, dd) rows -> channel-major attn_ct[phg2]
                    for j in range(2):
                        for p in range(P):
                            eng = remap_engines[(i * 6 + j * 3 + p) % 3]
                            eng.dma_start(
                                out=attn_ct[phg2][
                                    (2 * i + j) * 32:(2 * i + j + 1) * 32,
                                    p * TQ:(p + 1) * TQ],
                                in_=adiv[p * 32:(p + 1) * 32,
                                         j * TQ:(j + 1) * TQ])
            prev2 = prev1
            if it < NIT:
                prev1 = (e, g)

        # ================= equivariant LN (column form) =================
        # inorm = rsqrt(ssq/512 + eps), one Newton step; icol[t%128, th]
        acol = const.tile([128, 2], F32, tag="acol")
        nc.vector.tensor_scalar(
            out=acol[:], in0=ssq_col[:, :], scalar1=1.0 / HID, scalar2=EPS,
            op0=ALU.mult, op1=ALU.add)
        rcpa = const.tile([128, 2], F32, tag="rcpa")
        nc.vector.reciprocal_approx_fast(out=rcpa[:], in_=acol[:])
        r0t = const.tile([128, 2], F32, tag="r0t")
        nc.scalar.activation(r0t[:], rcpa[:], AF.Sqrt, bias=0.0, scale=1.0)
        tmp = const.tile([128, 2], F32, tag="tmpn")
        nc.vector.tensor_tensor(out=tmp[:], in0=r0t[:], in1=r0t[:],
                                op=ALU.mult)
        nc.vector.tensor_tensor(out=tmp[:], in0=tmp[:], in1=acol[:],
                                op=ALU.mult)
        nc.vector.tensor_scalar(
            out=tmp[:], in0=tmp[:], scalar1=-0.5, scalar2=1.5,
            op0=ALU.mult, op1=ALU.add)
        icol = const.tile([128, 2], F32, tag="icol")
        nc.vector.tensor_tensor(out=icol[:], in0=r0t[:], in1=tmp[:],
                                op=ALU.mult)
        if d_dbg is not None:
            for ci in range(4):
                nc.sync.dma_start(
                    out=d_dbg["attn"][ci * 128:(ci + 1) * 128, :],
                    in_=attn_ct[ci][:])

        # ================= out_proj =================
        for k in range(6):          # tiles: p = k // 2, t-half = k % 2
            p, th = k // 2, k % 2
            op = psum.tile([128, HID], F32, space="PSUM", tag="scores")
            for ci in range(4):
                nc.tensor.matmul(
                    out=op[:, :],
                    lhsT=attn_ct[ci][:, p * TQ + th * 128:
                                     p * TQ + (th + 1) * 128],
                    rhs=wT[:, ci * HID:(ci + 1) * HID],
                    start=(ci == 0), stop=(ci == 3))
            ot = drain.tile([128, HID], F32, tag="osb")
            if k % 2 == 0:
                nc.scalar.activation(ot[:], op[:, :], AF.Copy, bias=0.0,
                                     scale=icol[:, th:th + 1])
            else:
                nc.vector.tensor_scalar(
                    out=ot[:], in0=op[:, :], scalar1=icol[:, th:th + 1],
                    scalar2=None, op0=ALU.mult)
            nc.sync.dma_start(
                out=d_out[th * 128:th * 128 + 128, p, :],
                in_=ot[:])


def _host_prep(q, k, v, attn_bias, local_attention_weight, out_proj_w,
               ln_weight, outcell_index):
    """Pure layout marshalling on host -> per-core input dicts."""
    q = np.asarray(q, np.float32)
    k = np.asarray(k, np.float32)
    v = np.asarray(v, np.float32)
    attn_bias = np.asarray(attn_bias, np.float32)
    law = np.asarray(local_attention_weight, np.float32)
    out_proj_w = np.asarray(out_proj_w, np.float32)
    ln_weight = np.asarray(ln_weight, np.float32)
    idx = np.asarray(outcell_index).astype(np.int64)

    # (B,T,P,HID) -> (B, 96, H, T) with row j = p*32+dd
    def to_dT(x):
        return np.ascontiguousarray(
            x.reshape(B, T, P, H, D).transpose(0, 2, 4, 3, 1)
        ).reshape(B, P * D, H, T)

    qT = to_dT(q) * np.float32(SCALING)
    kT = to_dT(k)
    kTe = np.concatenate(
        [kT, np.take_along_axis(
            kT, idx[:, None, None, :].astype(np.int64), axis=3)], axis=3)
    lawT = np.ascontiguousarray(law.transpose(0, 2, 1))            # (B,S,T)
    biasT = np.ascontiguousarray(
        attn_bias.transpose(0, 3, 1, 2)).astype(np.float16)        # (B,S,H,T)
    vb = np.ascontiguousarray(
        v.reshape(B, T, P, H, D).transpose(0, 1, 3, 2, 4)
    ).reshape(B, T, P * HID).astype(np.float16)
    wln = (np.ascontiguousarray(out_proj_w.T) * ln_weight[:, None]
           ).astype(np.float32)
    wTimg = np.ascontiguousarray(
        wln.reshape(4, 128, HID).transpose(1, 0, 2)
    ).reshape(128, 4 * HID).astype(np.float16)
    vidx = idx.astype(np.int32).reshape(B, 2, 128, 1)

    in_maps = []
    for c in range(NCORES):
        b, th = c // 2, c % 2
        t0 = th * TQ
        lawc = np.ascontiguousarray(lawT[b, :, t0:t0 + TQ])
        lawc = np.ascontiguousarray(
            lawc.reshape(NST, 128, TQ).transpose(1, 0, 2)
        ).reshape(128, NST * TQ).astype(np.float16)
        # law4: per-st tile with the t-block replicated for the 4 heads
        law4c = np.ascontiguousarray(
            np.broadcast_to(
                lawc.reshape(128, NST, 1, TQ), (128, NST, 4, TQ))
        ).reshape(128, NST * 4 * TQ)
        # bias image: [(hg*NST+st)*128 + s%128, (i, t)]
        bc = biasT[b, :, :, t0:t0 + TQ]            # (S, H, TQ)
        bc = np.ascontiguousarray(
            bc.reshape(NST, 128, HG, 4, TQ).transpose(2, 0, 1, 3, 4)
        ).reshape(NIT * 128, 4 * TQ)
        in_maps.append(dict(
            biasT=bc,
            law4=law4c,
            qT=np.ascontiguousarray(
                qT[b, :, :, t0:t0 + TQ]).reshape(DH, H * TQ)
                .astype(np.float16),
            kTe=np.ascontiguousarray(kTe[b]).reshape(DH, H * S)
                .astype(np.float16),
            vb=np.ascontiguousarray(vb[b]),
            vidx=np.ascontiguousarray(vidx[b]),
            wT=wTimg,
            ident=np.eye(128, dtype=np.float16),
        ))
    return in_maps


def kernel(**inputs):
    global _CACHED_NC
    if _CACHED_NC is None:
        _CACHED_NC = build_nc()
    nc = _CACHED_NC
    in_maps = _host_prep(
        inputs["q"], inputs["k"], inputs["v"], inputs["attn_bias"],
        inputs["local_attention_weight"], inputs["out_proj_w"],
        inputs["ln_weight"], inputs["outcell_index"])
    res = run_bass_kernel_spmd(nc, in_maps, core_ids=list(range(NCORES)))
    out = np.empty((B, T, P, HID), np.float32)
    for c in range(NCORES):
        b, th = c // 2, c % 2
        out[b, th * TQ:(th + 1) * TQ] = res.results[c]["out"]
    return out
